# revision 4
# baseline (speedup 1.0000x reference)
"""Causal multi-head attention (B=2, L=2048, D=2048, H=16, rope theta=5e5)
on 8 Trainium2 NeuronCores.

Sharding: core c handles batch b = c//4 and heads 4*(c%4) .. 4*(c%4)+3.
Each core computes q/k/v projections for its 4 heads, rope, causal
attention, and a partial output projection y_part = attn_out @ Wo[rows].
Host sums the 4 partials per batch (row-parallel all-reduce) to produce y.

All matmuls run in float32r (TF32-like, 1 cycle/row on the PE) with fp32
accumulation. Softmax skips the max-subtraction (scores are ~N(0,1);
max over 134M samples < 7, exp is safe in fp32).
"""
import sys

sys.path.insert(0, "/opt/trn_rl_repo")

import numpy as np

import concourse.bass as bass
import concourse.tile as tile
from concourse import bacc, mybir
from concourse.bass_utils import run_bass_kernel_spmd

F32 = mybir.dt.float32
F32R = mybir.dt.float32r
EXP = mybir.ActivationFunctionType.Exp

P = 128          # partitions / head dim / tile edge
L = 2048         # sequence length
D = 2048         # model dim
NH = 4           # heads per core
BL = 512         # query block (l-block) size
NB = L // BL     # 4 l-blocks
ND = D // P      # 16 d chunks
NLT = BL // P    # 4 l-tiles per block
THETA = 500000.0
NEG = -1.0e30

_SWAP = []
for _i in range(16):
    _SWAP += [2 * _i + 1, 2 * _i]


def build_program():
    nc = bacc.Bacc("TRN2", target_bir_lowering=False, debug=False, num_devices=8)

    x_d = nc.dram_tensor("x", [L, D], F32R, kind="ExternalInput").ap()
    wq_d = nc.dram_tensor("wq", [D, NH * P], F32R, kind="ExternalInput").ap()
    wk_d = nc.dram_tensor("wk", [D, NH * P], F32R, kind="ExternalInput").ap()
    wv_d = nc.dram_tensor("wv", [D, NH * P], F32R, kind="ExternalInput").ap()
    wo_d = nc.dram_tensor("wo", [NH * P, D], F32R, kind="ExternalInput").ap()
    cos_d = nc.dram_tensor("cosT", [P, L], F32, kind="ExternalInput").ap()
    sig_d = nc.dram_tensor("sigT", [P, L], F32, kind="ExternalInput").ap()
    mask_d = nc.dram_tensor("masks", [P, NLT, BL], F32, kind="ExternalInput").ap()
    id_d = nc.dram_tensor("iden", [P, P], F32R, kind="ExternalInput").ap()
    ones_d = nc.dram_tensor("ones", [P, 1], F32R, kind="ExternalInput").ap()
    y_d = nc.dram_tensor("y", [L, D], F32, kind="ExternalOutput").ap()

    with tile.TileContext(nc) as tc:
        with (
            tc.tile_pool(name="const", bufs=1) as cpool,
            tc.tile_pool(name="persist", bufs=1) as ppool,
            tc.tile_pool(name="work", bufs=2) as wpool,
            tc.tile_pool(name="ps", bufs=8, space="PSUM") as ps,
        ):
            t_id = cpool.tile([P, P], F32R, tag="t_id")
            t_ones = cpool.tile([P, 1], F32R, tag="t_ones")
            t_mask = cpool.tile([P, NLT, BL], F32, tag="t_mask")
            nc.sync.dma_start(t_id[:], id_d[:])
            nc.sync.dma_start(t_ones[:], ones_d[:])
            nc.sync.dma_start(t_mask[:], mask_d[:])

            # persistent full-L tensors (per head slices along free axis)
            kT = ppool.tile([P, NH * L], F32R, tag="kT")     # [dh, h*L + l]
            vv = ppool.tile([P, ND * NH * P], F32R, tag="vv")  # [l%128, j0*512 + h*128 + dh]
            qTs = ppool.tile([P, NH * BL], F32R, tag="qTs")  # current block [dh, h*BL + l]

            for B in range(NB):
                l0 = B * BL
                # ---- load x rows and rope tables for this block ----
                t_cos = wpool.tile([P, BL], F32, tag="t_cos")
                t_sig = wpool.tile([P, BL], F32, tag="t_sig")
                nc.sync.dma_start(t_cos[:], cos_d[:, l0 : l0 + BL])
                nc.sync.dma_start(t_sig[:], sig_d[:, l0 : l0 + BL])

                # ---- transpose x[l0:l0+BL, :] -> xT (chunked by d) ----
                xT = wpool.tile([P, ND * BL], F32R, tag="xT", bufs=1)  # [d%128, d_chunk*BL + l]
                for ltp in range(2):  # l-tile pairs
                    xrs = []
                    for lt in (2 * ltp, 2 * ltp + 1):
                        xr = wpool.tile([P, D], F32R, tag="xr")
                        nc.sync.dma_start(xr[:], x_d[l0 + lt * P : l0 + (lt + 1) * P, :])
                        xrs.append(xr)
                    for d in range(ND):
                        pt = ps.tile([P, 2 * P], F32R, tag="ps")
                        for i, lt in enumerate((2 * ltp, 2 * ltp + 1)):
                            nc.tensor.transpose(
                                pt[:, i * P : (i + 1) * P],
                                xrs[i][:, d * P : (d + 1) * P],
                                t_id[:],
                            )
                        nc.scalar.copy(
                            xT[:, d * BL + 2 * ltp * P : d * BL + (2 * ltp + 2) * P],
                            pt[:],
                        )

                # ---- q / k projections + rope ----
                for which, w_dram in (("q", wq_d), ("k", wk_d)):
                    psqk = [
                        ps.tile([P, BL], F32, tag="ps", name=f"psqk{B}{which}{h}")
                        for h in range(NH)
                    ]
                    for d in range(ND):
                        tw = wpool.tile([P, NH * P], F32R, tag="tw", bufs=3, name=f"tw{B}{which}{d}")
                        nc.sync.dma_start(tw[:], w_dram[d * P : (d + 1) * P, :])
                        for h in range(NH):
                            nc.tensor.matmul(
                                psqk[h][:],
                                tw[:, h * P : (h + 1) * P],
                                xT[:, d * BL : (d + 1) * BL],
                                start=(d == 0),
                                stop=(d == ND - 1),
                            )
                    for h in range(NH):
                        tsw = wpool.tile([P, BL], F32, tag="ropetmp", bufs=3)
                        nc.vector.stream_shuffle(tsw[:], psqk[h][:], _SWAP)
                        tt2 = wpool.tile([P, BL], F32, tag="ropetmp", bufs=3)
                        nc.vector.tensor_mul(tt2[:], tsw[:], t_sig[:])
                        tt3 = wpool.tile([P, BL], F32, tag="ropetmp", bufs=3)
                        nc.vector.tensor_mul(tt3[:], psqk[h][:], t_cos[:])
                        if which == "q":
                            out_sl = qTs[:, h * BL : (h + 1) * BL]
                        else:
                            out_sl = kT[:, h * L + l0 : h * L + l0 + BL]
                        nc.vector.tensor_add(out_sl, tt3[:], tt2[:])

                # ---- v projection ----
                psv = [
                    ps.tile([P, NH * P], F32, tag="ps", name=f"psv{B}{lt}")
                    for lt in range(NLT)
                ]
                for d in range(ND):
                    tw = wpool.tile([P, NH * P], F32R, tag="tw", bufs=3, name=f"twv{B}{d}")
                    nc.sync.dma_start(tw[:], wv_d[d * P : (d + 1) * P, :])
                    for lt in range(NLT):
                        nc.tensor.matmul(
                            psv[lt][:],
                            xT[:, d * BL + lt * P : d * BL + (lt + 1) * P],
                            tw[:],
                            start=(d == 0),
                            stop=(d == ND - 1),
                        )
                for lt in range(NLT):
                    j0 = NLT * B + lt
                    nc.scalar.copy(vv[:, j0 * NH * P : (j0 + 1) * NH * P], psv[lt][:])

                # ---- causal attention for this query block ----
                nk = NLT * B + NLT  # valid key chunks
                rcol = wpool.tile([NH, BL], F32, tag="rcol")
                psos = []
                for h in range(NH):
                    pso = ps.tile([P, BL], F32, tag="ps")
                    psos.append(pso)
                    prs = ps.tile([1, BL], F32, tag="ps")
                    for j0 in range(nk):
                        pss = ps.tile([P, BL], F32, tag="ps")
                        nc.tensor.matmul(
                            pss[:],
                            kT[:, h * L + j0 * P : h * L + (j0 + 1) * P],
                            qTs[:, h * BL : (h + 1) * BL],
                            start=True,
                            stop=True,
                        )
                        at = wpool.tile([P, BL], F32R, tag="at", bufs=3)
                        if j0 >= NLT * B:  # diagonal tile: causal mask
                            sm = wpool.tile([P, BL], F32, tag="sm")
                            nc.vector.tensor_add(
                                sm[:], pss[:], t_mask[:, j0 - NLT * B, :]
                            )
                            nc.scalar.activation(at[:], sm[:], EXP)
                        else:
                            nc.scalar.activation(at[:], pss[:], EXP)
                        nc.tensor.matmul(
                            psos[h][:],
                            vv[:, j0 * NH * P + h * P : j0 * NH * P + (h + 1) * P],
                            at[:],
                            start=(j0 == 0),
                            stop=(j0 == nk - 1),
                        )
                        nc.tensor.matmul(
                            prs[:],
                            t_ones[:],
                            at[:],
                            start=(j0 == 0),
                            stop=(j0 == nk - 1),
                        )
                    rtmp = wpool.tile([1, BL], F32, tag="rtmp")
                    nc.vector.tensor_copy(rtmp[:], prs[:])
                    nc.sync.dma_start(rcol[h : h + 1, :], rtmp[:])

                rcinv = wpool.tile([NH, BL], F32, tag="rcinv")
                nc.vector.reciprocal(rcinv[:], rcol[:])
                oTb = wpool.tile([P, NH * BL], F32R, tag="oTb", bufs=1)
                for h in range(NH):
                    tmp0 = wpool.tile([1, BL], F32, tag="tmp0")
                    nc.sync.dma_start(tmp0[:], rcinv[h : h + 1, :])
                    rb = wpool.tile([P, BL], F32, tag="rb")
                    nc.gpsimd.partition_broadcast(rb[:], tmp0[:])
                    nc.vector.tensor_mul(
                        oTb[:, h * BL : (h + 1) * BL], psos[h][:], rb[:]
                    )

                # ---- partial output projection for this block's rows ----
                for Db in range(NB):
                    tws = []
                    for h in range(NH):
                        two = wpool.tile([P, BL], F32R, tag="two", bufs=6, name=f"two{B}{Db}{h}")
                        nc.sync.dma_start(
                            two[:], wo_d[h * P : (h + 1) * P, Db * BL : (Db + 1) * BL]
                        )
                        tws.append(two)
                    for lt in range(NLT):
                        psy = ps.tile([P, BL], F32, tag="ps")
                        for h in range(NH):
                            nc.tensor.matmul(
                                psy[:],
                                oTb[:, h * BL + lt * P : h * BL + (lt + 1) * P],
                                tws[h][:],
                                start=(h == 0),
                                stop=(h == NH - 1),
                            )
                        ye = wpool.tile([P, BL], F32, tag="ye")
                        nc.scalar.copy(ye[:], psy[:])
                        nc.sync.dma_start(
                            y_d[l0 + lt * P : l0 + (lt + 1) * P, Db * BL : (Db + 1) * BL],
                            ye[:],
                        )
    nc.compile()
    return nc


_NC_CACHE = None


def _get_program():
    global _NC_CACHE
    if _NC_CACHE is None:
        _NC_CACHE = build_program()
    return _NC_CACHE


def _host_tables():
    hd = P  # head dim
    i = np.arange(hd // 2, dtype=np.float64)
    invf = THETA ** (-2.0 * i / hd)  # [64]
    t = np.arange(L, dtype=np.float64)
    ang = np.outer(invf, t)  # [64, L]
    cos = np.cos(ang)
    sin = np.sin(ang)
    cosT = np.repeat(cos, 2, axis=0).astype(np.float32)  # [128, L]
    sigT = np.empty((P, L), dtype=np.float32)
    sigT[0::2] = -sin
    sigT[1::2] = sin

    masks = np.empty((P, NLT, BL), dtype=np.float32)
    j = np.arange(P)[:, None]
    ii = np.arange(BL)[None, :]
    for off in range(NLT):
        masks[:, off, :] = np.where(ii >= off * P + j, 0.0, NEG)
    return cosT, sigT, masks


def kernel(x, Wq, Wk, Wv, Wo):
    x = np.asarray(x, dtype=np.float32)
    Wq = np.asarray(Wq, dtype=np.float32)
    Wk = np.asarray(Wk, dtype=np.float32)
    Wv = np.asarray(Wv, dtype=np.float32)
    Wo = np.asarray(Wo, dtype=np.float32)
    Bsz = x.shape[0]

    nc = _get_program()
    cosT, sigT, masks = _host_tables()
    iden = np.eye(P, dtype=np.float32)
    ones = np.ones((P, 1), dtype=np.float32)
    scale = 1.0 / np.sqrt(float(P))

    in_maps = []
    for c in range(8):
        b = c // 4
        g = c % 4
        hs = slice(NH * g, NH * g + NH)
        in_maps.append(
            {
                "x": np.ascontiguousarray(x[b]),
                "wq": np.ascontiguousarray(
                    Wq[:, hs, :].reshape(D, NH * P) * scale
                ),
                "wk": np.ascontiguousarray(Wk[:, hs, :].reshape(D, NH * P)),
                "wv": np.ascontiguousarray(Wv[:, hs, :].reshape(D, NH * P)),
                "wo": np.ascontiguousarray(Wo[NH * P * g : NH * P * (g + 1), :]),
                "cosT": cosT,
                "sigT": sigT,
                "masks": masks,
                "iden": iden,
                "ones": ones,
            }
        )

    res = run_bass_kernel_spmd(nc, in_maps, list(range(8)))
    y = np.zeros((Bsz, L, D), dtype=np.float32)
    for c in range(8):
        y[c // 4] += res.results[c]["y"]
    return y


# revision 7
# speedup vs baseline: 161.6761x; 161.6761x over previous
"""Causal multi-head attention (B=2, L=2048, D=2048, H=16, rope theta=5e5)
on 8 Trainium2 NeuronCores.

Sharding: core c handles batch b = c//4 and heads 4*(c%4) .. 4*(c%4)+3.
Each core computes q/k/v projections for its 4 heads, rope, causal
attention, and a partial output projection y_part = attn_out @ Wo[rows].
Host sums the 4 partials per batch (row-parallel all-reduce) to produce y.

All matmuls run in float32r (TF32-like, 1 cycle/row on the PE) with fp32
accumulation. Softmax skips the max-subtraction (scores are ~N(0,1);
max over 134M samples < 7, exp is safe in fp32).
"""
import sys

sys.path.insert(0, "/opt/trn_rl_repo")

import numpy as np

import concourse.bass as bass
import concourse.tile as tile
from concourse import bacc, mybir
from concourse.bass_utils import run_bass_kernel_spmd

F32 = mybir.dt.float32
F32R = mybir.dt.float32r
EXP = mybir.ActivationFunctionType.Exp

P = 128          # partitions / head dim / tile edge
L = 2048         # sequence length
D = 2048         # model dim
NH = 4           # heads per core
BL = 512         # query block (l-block) size
NB = L // BL     # 4 l-blocks
ND = D // P      # 16 d chunks
NLT = BL // P    # 4 l-tiles per block
THETA = 500000.0
NEG = -1.0e30

_SWAP = []
for _i in range(16):
    _SWAP += [2 * _i + 1, 2 * _i]


def build_program(nrep=1):
    nc = bacc.Bacc("TRN2", target_bir_lowering=False, debug=False, num_devices=8)

    x_d = nc.dram_tensor("x", [L, D], F32R, kind="ExternalInput").ap()
    wq_d = nc.dram_tensor("wq", [D, NH * P], F32R, kind="ExternalInput").ap()
    wk_d = nc.dram_tensor("wk", [D, NH * P], F32R, kind="ExternalInput").ap()
    wv_d = nc.dram_tensor("wv", [D, NH * P], F32R, kind="ExternalInput").ap()
    wo_d = nc.dram_tensor("wo", [NH * P, D], F32R, kind="ExternalInput").ap()
    cos_d = nc.dram_tensor("cosT", [P, L], F32, kind="ExternalInput").ap()
    sig_d = nc.dram_tensor("sigT", [P, L], F32, kind="ExternalInput").ap()
    mask_d = nc.dram_tensor("masks", [P, NLT, BL], F32, kind="ExternalInput").ap()
    id_d = nc.dram_tensor("iden", [P, P], F32R, kind="ExternalInput").ap()
    ones_d = nc.dram_tensor("ones", [P, 1], F32R, kind="ExternalInput").ap()
    y_d = nc.dram_tensor("y", [L, D], F32, kind="ExternalOutput").ap()

    with tile.TileContext(nc) as tc:
        with (
            tc.tile_pool(name="const", bufs=1) as cpool,
            tc.tile_pool(name="persist", bufs=1) as ppool,
            tc.tile_pool(name="work", bufs=2) as wpool,
            tc.tile_pool(name="ps", bufs=8, space="PSUM") as ps,
        ):
            t_id = cpool.tile([P, P], F32R, tag="t_id")
            t_ones = cpool.tile([P, 1], F32R, tag="t_ones")
            t_mask = cpool.tile([P, NLT, BL], F32, tag="t_mask")
            nc.sync.dma_start(t_id[:], id_d[:])
            nc.sync.dma_start(t_ones[:], ones_d[:])
            nc.sync.dma_start(t_mask[:], mask_d[:])

            # persistent full-L tensors (per head slices along free axis)
            kT = ppool.tile([P, NH * L], F32R, tag="kT")     # [dh, h*L + l]
            vv = ppool.tile([P, ND * NH * P], F32R, tag="vv")  # [l%128, j0*512 + h*128 + dh]
            qTs = ppool.tile([P, NH * BL], F32R, tag="qTs")  # current block [dh, h*BL + l]

            import contextlib

            rep_ctx = tc.For_i(0, nrep, 1) if nrep > 1 else contextlib.nullcontext()
            with rep_ctx:
                _build_body(nc, tc, locals())
    nc.compile()
    return nc


def _build_body(nc, tc, env):
    t_id, t_ones, t_mask = env["t_id"], env["t_ones"], env["t_mask"]
    kT, vv, qTs = env["kT"], env["vv"], env["qTs"]
    wpool, ps = env["wpool"], env["ps"]
    x_d, wq_d, wk_d, wv_d, wo_d = (
        env["x_d"], env["wq_d"], env["wk_d"], env["wv_d"], env["wo_d"]
    )
    cos_d, sig_d, y_d = env["cos_d"], env["sig_d"], env["y_d"]
    if True:
            for B in range(NB):
                l0 = B * BL
                # ---- load x rows and rope tables for this block ----
                t_cos = wpool.tile([P, BL], F32, tag="t_cos")
                t_sig = wpool.tile([P, BL], F32, tag="t_sig")
                nc.sync.dma_start(t_cos[:], cos_d[:, l0 : l0 + BL])
                nc.sync.dma_start(t_sig[:], sig_d[:, l0 : l0 + BL])

                # ---- transpose x[l0:l0+BL, :] -> xT (chunked by d) ----
                xT = wpool.tile([P, ND * BL], F32R, tag="xT", bufs=1)  # [d%128, d_chunk*BL + l]
                for ltp in range(2):  # l-tile pairs
                    xrs = []
                    for lt in (2 * ltp, 2 * ltp + 1):
                        xr = wpool.tile([P, D], F32R, tag="xr")
                        nc.sync.dma_start(xr[:], x_d[l0 + lt * P : l0 + (lt + 1) * P, :])
                        xrs.append(xr)
                    for d in range(ND):
                        pt = ps.tile([P, 2 * P], F32R, tag="ps")
                        for i, lt in enumerate((2 * ltp, 2 * ltp + 1)):
                            nc.tensor.transpose(
                                pt[:, i * P : (i + 1) * P],
                                xrs[i][:, d * P : (d + 1) * P],
                                t_id[:],
                            )
                        nc.scalar.copy(
                            xT[:, d * BL + 2 * ltp * P : d * BL + (2 * ltp + 2) * P],
                            pt[:],
                        )

                # ---- q / k projections + rope ----
                for which, w_dram in (("q", wq_d), ("k", wk_d)):
                    psqk = [
                        ps.tile([P, BL], F32, tag="ps", name=f"psqk{B}{which}{h}")
                        for h in range(NH)
                    ]
                    for d in range(ND):
                        tw = wpool.tile([P, NH * P], F32R, tag="tw", bufs=3, name=f"tw{B}{which}{d}")
                        nc.sync.dma_start(tw[:], w_dram[d * P : (d + 1) * P, :])
                        for h in range(NH):
                            nc.tensor.matmul(
                                psqk[h][:],
                                tw[:, h * P : (h + 1) * P],
                                xT[:, d * BL : (d + 1) * BL],
                                start=(d == 0),
                                stop=(d == ND - 1),
                            )
                    for h in range(NH):
                        tsw = wpool.tile([P, BL], F32, tag="ropetmp", bufs=3)
                        nc.vector.stream_shuffle(tsw[:], psqk[h][:], _SWAP)
                        tt2 = wpool.tile([P, BL], F32, tag="ropetmp", bufs=3)
                        nc.vector.tensor_mul(tt2[:], tsw[:], t_sig[:])
                        tt3 = wpool.tile([P, BL], F32, tag="ropetmp", bufs=3)
                        nc.vector.tensor_mul(tt3[:], psqk[h][:], t_cos[:])
                        if which == "q":
                            out_sl = qTs[:, h * BL : (h + 1) * BL]
                        else:
                            out_sl = kT[:, h * L + l0 : h * L + l0 + BL]
                        nc.vector.tensor_add(out_sl, tt3[:], tt2[:])

                # ---- v projection ----
                psv = [
                    ps.tile([P, NH * P], F32, tag="ps", name=f"psv{B}{lt}")
                    for lt in range(NLT)
                ]
                for d in range(ND):
                    tw = wpool.tile([P, NH * P], F32R, tag="tw", bufs=3, name=f"twv{B}{d}")
                    nc.sync.dma_start(tw[:], wv_d[d * P : (d + 1) * P, :])
                    for lt in range(NLT):
                        nc.tensor.matmul(
                            psv[lt][:],
                            xT[:, d * BL + lt * P : d * BL + (lt + 1) * P],
                            tw[:],
                            start=(d == 0),
                            stop=(d == ND - 1),
                        )
                for lt in range(NLT):
                    j0 = NLT * B + lt
                    nc.scalar.copy(vv[:, j0 * NH * P : (j0 + 1) * NH * P], psv[lt][:])

                # ---- causal attention for this query block ----
                nk = NLT * B + NLT  # valid key chunks
                rcol = wpool.tile([NH, BL], F32, tag="rcol")
                psos = []
                for h in range(NH):
                    pso = ps.tile([P, BL], F32, tag="ps")
                    psos.append(pso)
                    prs = ps.tile([1, BL], F32, tag="ps")
                    for j0 in range(nk):
                        pss = ps.tile([P, BL], F32, tag="ps")
                        nc.tensor.matmul(
                            pss[:],
                            kT[:, h * L + j0 * P : h * L + (j0 + 1) * P],
                            qTs[:, h * BL : (h + 1) * BL],
                            start=True,
                            stop=True,
                        )
                        at = wpool.tile([P, BL], F32R, tag="at", bufs=3)
                        if j0 >= NLT * B:  # diagonal tile: causal mask
                            sm = wpool.tile([P, BL], F32, tag="sm")
                            nc.vector.tensor_add(
                                sm[:], pss[:], t_mask[:, j0 - NLT * B, :]
                            )
                            nc.scalar.activation(at[:], sm[:], EXP)
                        else:
                            nc.scalar.activation(at[:], pss[:], EXP)
                        nc.tensor.matmul(
                            psos[h][:],
                            vv[:, j0 * NH * P + h * P : j0 * NH * P + (h + 1) * P],
                            at[:],
                            start=(j0 == 0),
                            stop=(j0 == nk - 1),
                        )
                        nc.tensor.matmul(
                            prs[:],
                            t_ones[:],
                            at[:],
                            start=(j0 == 0),
                            stop=(j0 == nk - 1),
                        )
                    rtmp = wpool.tile([1, BL], F32, tag="rtmp")
                    nc.vector.tensor_copy(rtmp[:], prs[:])
                    nc.sync.dma_start(rcol[h : h + 1, :], rtmp[:])

                rcinv = wpool.tile([NH, BL], F32, tag="rcinv")
                nc.vector.reciprocal(rcinv[:], rcol[:])
                oTb = wpool.tile([P, NH * BL], F32R, tag="oTb", bufs=1)
                for h in range(NH):
                    tmp0 = wpool.tile([1, BL], F32, tag="tmp0")
                    nc.sync.dma_start(tmp0[:], rcinv[h : h + 1, :])
                    rb = wpool.tile([P, BL], F32, tag="rb")
                    nc.gpsimd.partition_broadcast(rb[:], tmp0[:])
                    nc.vector.tensor_mul(
                        oTb[:, h * BL : (h + 1) * BL], psos[h][:], rb[:]
                    )

                # ---- partial output projection for this block's rows ----
                for Db in range(NB):
                    tws = []
                    for h in range(NH):
                        two = wpool.tile([P, BL], F32R, tag="two", bufs=6, name=f"two{B}{Db}{h}")
                        nc.sync.dma_start(
                            two[:], wo_d[h * P : (h + 1) * P, Db * BL : (Db + 1) * BL]
                        )
                        tws.append(two)
                    for lt in range(NLT):
                        psy = ps.tile([P, BL], F32, tag="ps")
                        for h in range(NH):
                            nc.tensor.matmul(
                                psy[:],
                                oTb[:, h * BL + lt * P : h * BL + (lt + 1) * P],
                                tws[h][:],
                                start=(h == 0),
                                stop=(h == NH - 1),
                            )
                        ye = wpool.tile([P, BL], F32, tag="ye")
                        nc.scalar.copy(ye[:], psy[:])
                        nc.sync.dma_start(
                            y_d[l0 + lt * P : l0 + (lt + 1) * P, Db * BL : (Db + 1) * BL],
                            ye[:],
                        )


_NC_CACHE = None


def _get_program():
    global _NC_CACHE
    if _NC_CACHE is None:
        _NC_CACHE = build_program()
    return _NC_CACHE


def _host_tables():
    hd = P  # head dim
    i = np.arange(hd // 2, dtype=np.float64)
    invf = THETA ** (-2.0 * i / hd)  # [64]
    t = np.arange(L, dtype=np.float64)
    ang = np.outer(invf, t)  # [64, L]
    cos = np.cos(ang)
    sin = np.sin(ang)
    cosT = np.repeat(cos, 2, axis=0).astype(np.float32)  # [128, L]
    sigT = np.empty((P, L), dtype=np.float32)
    sigT[0::2] = -sin
    sigT[1::2] = sin

    masks = np.empty((P, NLT, BL), dtype=np.float32)
    j = np.arange(P)[:, None]
    ii = np.arange(BL)[None, :]
    for off in range(NLT):
        masks[:, off, :] = np.where(ii >= off * P + j, 0.0, NEG)
    return cosT, sigT, masks


def kernel(x, Wq, Wk, Wv, Wo):
    x = np.asarray(x, dtype=np.float32)
    Wq = np.asarray(Wq, dtype=np.float32)
    Wk = np.asarray(Wk, dtype=np.float32)
    Wv = np.asarray(Wv, dtype=np.float32)
    Wo = np.asarray(Wo, dtype=np.float32)
    Bsz = x.shape[0]

    nc = _get_program()
    cosT, sigT, masks = _host_tables()
    iden = np.eye(P, dtype=np.float32)
    ones = np.ones((P, 1), dtype=np.float32)
    scale = 1.0 / np.sqrt(float(P))

    in_maps = []
    for c in range(8):
        b = c // 4
        g = c % 4
        hs = slice(NH * g, NH * g + NH)
        in_maps.append(
            {
                "x": np.ascontiguousarray(x[b]),
                "wq": np.ascontiguousarray(
                    Wq[:, hs, :].reshape(D, NH * P) * scale
                ),
                "wk": np.ascontiguousarray(Wk[:, hs, :].reshape(D, NH * P)),
                "wv": np.ascontiguousarray(Wv[:, hs, :].reshape(D, NH * P)),
                "wo": np.ascontiguousarray(Wo[NH * P * g : NH * P * (g + 1), :]),
                "cosT": cosT,
                "sigT": sigT,
                "masks": masks,
                "iden": iden,
                "ones": ones,
            }
        )

    res = run_bass_kernel_spmd(nc, in_maps, list(range(8)))
    y = np.zeros((Bsz, L, D), dtype=np.float32)
    for c in range(8):
        y[c // 4] += res.results[c]["y"]
    return y
